# revision 7
# baseline (speedup 1.0000x reference)
"""Trainium2 Bass kernel for the attention-mechanism problem.

Computation (per batch b):
    a3     = tanh(h @ w + bias)            # [L, HID]
    scores = a3 @ u                        # [L, SEQ]
    a      = softmax(scores, axis=L)       # softmax over L (dim 0 of [L, SEQ])
    S      = a.T @ h                       # [SEQ, IN]

Shapes: h [64, 512, 1024] f32, w [1024, 1024], u [1024, 512], b [1].
Sharding: data-parallel over batch across 8 NeuronCores (8 batches/core),
params replicated. No collectives.

Precision strategy: the softmax input (scores ~ +-60) is extremely
sensitive to matmul error, so steps 1-2 run the PE in float32r mode
(full speed at moving-dim 512, near-fp32 accuracy).  h is transposed for
step 1 via an fp16 hi+lo split (h = hi + lo exactly to ~2^-22) so the
2-byte DMA xbar transpose can be used, recombined to f32 by DVE adds.
Step 3 (the final weighted average, |a|<=1) runs in fp16.

All xbar transposes write fully-contiguous SBUF tiles (sliced/strided
transpose destinations are known-broken on HW); sources are SBUF slices
that optimize to 2D.
"""

import sys

for _p in ("/opt/trn_rl_repo", "/root/.axon_site/_ro/trn_rl_repo"):
    if _p not in sys.path:
        sys.path.insert(0, _p)

import numpy as np

import concourse.bass as bass  # noqa: F401
import concourse.tile as tile
from concourse import bacc, mybir
from concourse.bass_utils import run_bass_kernel_spmd

B, L, IN, HID, SEQ = 64, 512, 1024, 1024, 512
NCORES = 8
BB = B // NCORES  # batches per core

FP32 = mybir.dt.float32
FP32R = mybir.dt.float32r
FP16 = mybir.dt.float16
AX = mybir.AxisListType.X
AF = mybir.ActivationFunctionType

P = 128
KI = IN // P    # 8 IN chunks
KH = HID // P   # 8 HID chunks
CL = L // P     # 4 L chunks
CS = SEQ // P   # 4 SEQ chunks
NH = IN // 512  # 2 IN halves for step3 moving operand


def build_nc() -> bacc.Bacc:
    nc = bacc.Bacc("TRN2", target_bir_lowering=False, debug=False)

    h = nc.declare_dram_parameter("h", [BB, L, IN], FP32, isOutput=False)
    w = nc.declare_dram_parameter("w", [IN, HID], FP32, isOutput=False)
    u = nc.declare_dram_parameter("u", [HID, SEQ], FP32, isOutput=False)
    b = nc.declare_dram_parameter("b", [1], FP32, isOutput=False)
    out = nc.declare_dram_parameter("out", [BB, SEQ, IN], FP32, isOutput=True)

    with tile.TileContext(nc) as tc:
        with (
            tc.tile_pool(name="singles", bufs=1) as singles,
            tc.tile_pool(name="hf", bufs=2) as hf_pool,
            tc.tile_pool(name="hhi", bufs=2) as hhi_pool,
            tc.tile_pool(name="hlo", bufs=1) as hlo_pool,
            tc.tile_pool(name="htp", bufs=4) as htp_pool,
            tc.tile_pool(name="htf", bufs=2) as htf_pool,
            tc.tile_pool(name="a3", bufs=1) as a3_pool,
            tc.tile_pool(name="at", bufs=1) as at_pool,
            tc.tile_pool(name="av", bufs=8) as a_pool,
            tc.tile_pool(name="so", bufs=2) as s_pool,
            tc.tile_pool(name="small", bufs=4) as small_pool,
            tc.tile_pool(name="ps1", bufs=3, space="PSUM") as ps1_pool,
            tc.tile_pool(name="ps2", bufs=2, space="PSUM") as ps2_pool,
            tc.tile_pool(name="ps3", bufs=3, space="PSUM") as ps3_pool,
        ):
            # ---- params (all kept f32; PE consumes them as f32r) ----
            b_sb = singles.tile([P, 1], FP32)
            nc.sync.dma_start(out=b_sb, in_=b.ap().to_broadcast((P, 1)))

            # w_r[p, c, m] = w[c*128+p, m]  (K=IN on partitions), rounded to f32r
            # for the PE's fast fp32 mode.  DMA-stage f32 then round-copy: the
            # BIR verifier requires f32r-consumed tensors to be written by an
            # instruction that rounds to f32r.
            w_r = singles.tile([P, KI, HID], FP32R)
            w_re = w.ap().rearrange("(c p) m -> p c m", p=P)
            for i in range(2):
                wstage = hf_pool.tile([P, CL, IN], FP32, tag="h_f", name="wstage")
                nc.sync.dma_start(out=wstage, in_=w_re[:, 4 * i : 4 * i + 4, :])
                nc.vector.tensor_copy(out=w_r[:, 4 * i : 4 * i + 4, :], in_=wstage)

            # u_r[p, c, s] = u[c*128+p, s]  (K=HID on partitions)
            u_r = singles.tile([P, KH, SEQ], FP32R)
            ustage = hf_pool.tile([P, KH, SEQ], FP32, tag="h_f", name="ustage")
            nc.sync.dma_start(
                out=ustage, in_=u.ap().rearrange("(c p) s -> p c s", p=P)
            )
            nc.vector.tensor_copy(out=u_r, in_=ustage)

            for ib in range(BB):
                # ---- load h batch, split into fp16 hi+lo ----
                h_f = hf_pool.tile([P, CL, IN], FP32, tag="h_f")
                nc.sync.dma_start(
                    out=h_f, in_=h[ib].rearrange("(c p) i -> p c i", p=P)
                )
                h_hi = hhi_pool.tile([P, CL, IN], FP16)
                nc.vector.tensor_copy(out=h_hi, in_=h_f)
                h_lo = hlo_pool.tile([P, CL, IN], FP16)
                nc.vector.tensor_sub(h_lo, h_f, h_hi)

                # ---- hT = (h_hi + h_lo).T via per-L-chunk xbar transposes ----
                hT = htf_pool.tile([P, KI, L], FP32R)  # hT[p, c, l] = h[l, c*128+p]
                for lc in range(CL):
                    t_hi = htp_pool.tile([P, KI, P], FP16, tag="htp", name="t_hi")
                    nc.sync.dma_start_transpose(out=t_hi, in_=h_hi[:, lc, :])
                    t_lo = htp_pool.tile([P, KI, P], FP16, tag="htp", name="t_lo")
                    nc.sync.dma_start_transpose(out=t_lo, in_=h_lo[:, lc, :])
                    nc.vector.tensor_add(
                        hT[:, :, lc * P : (lc + 1) * P], t_hi, t_lo
                    )

                # ---- step1: a3T[m, l] = tanh(sum_k w[k, m] * hT[k, l] + b) ----
                a3T = a3_pool.tile([P, KH, L], FP32R)
                for mh in range(KH):
                    ps1 = ps1_pool.tile([P, L], FP32)
                    for kc in range(KI):
                        nc.tensor.matmul(
                            ps1,
                            lhsT=w_r[:, kc, mh * P : (mh + 1) * P],
                            rhs=hT[:, kc, :],
                            start=(kc == 0),
                            stop=(kc == KI - 1),
                        )
                    nc.scalar.activation(
                        out=a3T[:, mh, :], in_=ps1, func=AF.Tanh, bias=b_sb, scale=1.0
                    )

                # ---- step2: scoresT[s, l] = sum_k u[k, s] * a3T[k, l]; softmax over l ----
                sums = small_pool.tile([P, CS], FP32)
                aT = at_pool.tile([P, CS, L], FP16)  # exp(scoresT - max), unnormalized
                for sc in range(CS):
                    ps2 = ps2_pool.tile([P, L], FP32)
                    for kh in range(KH):
                        nc.tensor.matmul(
                            ps2,
                            lhsT=u_r[:, kh, sc * P : (sc + 1) * P],
                            rhs=a3T[:, kh, :],
                            start=(kh == 0),
                            stop=(kh == KH - 1),
                        )
                    negmax = small_pool.tile([P, 1], FP32, tag="negmax")
                    nc.vector.reduce_max(out=negmax, in_=ps2, axis=AX, negate=True)
                    nc.scalar.activation(
                        out=aT[:, sc, :],
                        in_=ps2,
                        func=AF.Exp,
                        bias=negmax,
                        scale=1.0,
                        accum_out=sums[:, sc : sc + 1],
                    )
                recips = small_pool.tile([P, CS], FP32)
                nc.vector.reciprocal(out=recips, in_=sums)

                # ---- a pieces: transpose each SEQ chunk of aT ----
                # a_sc[p, c, j] = a[c*128+p, sc*128+j]  (L on partitions)
                a_pieces = []
                for sc in range(CS):
                    a_sc = a_pool.tile([P, CL, P], FP16, tag="a_sc", name="a_sc")
                    nc.sync.dma_start_transpose(out=a_sc, in_=aT[:, sc, :])
                    a_pieces.append(a_sc)

                # ---- step3: S[s, i] = (1/D[s]) * sum_l aT[s, l] * h[l, i] ----
                for mc in range(CS):
                    for ih in range(NH):
                        ps3 = ps3_pool.tile([P, 512], FP32)
                        for kc in range(CL):
                            nc.tensor.matmul(
                                ps3,
                                lhsT=a_pieces[mc][:, kc, :],
                                rhs=h_hi[:, kc, ih * 512 : (ih + 1) * 512],
                                start=(kc == 0),
                                stop=(kc == CL - 1),
                            )
                        s_half = s_pool.tile([P, 512], FP32, tag="s_half", bufs=4)
                        nc.vector.tensor_scalar_mul(
                            out=s_half,
                            in0=ps3,
                            scalar1=recips[:, mc : mc + 1],
                        )
                        nc.sync.dma_start(
                            out=out[ib]
                            .rearrange("(c p) i -> p c i", p=P)[
                                :, mc, ih * 512 : (ih + 1) * 512
                            ],
                            in_=s_half,
                        )

    nc.finalize()
    return nc


_NC_CACHE = None


def kernel(h: np.ndarray, w: np.ndarray, u: np.ndarray, b: np.ndarray, **_ignored):
    global _NC_CACHE
    if _NC_CACHE is None:
        _NC_CACHE = build_nc()
    nc = _NC_CACHE

    h = np.ascontiguousarray(h, dtype=np.float32)
    w = np.ascontiguousarray(w, dtype=np.float32)
    u = np.ascontiguousarray(u, dtype=np.float32)
    b = np.ascontiguousarray(b, dtype=np.float32)

    in_maps = [
        {"h": h[i * BB : (i + 1) * BB], "w": w, "u": u, "b": b}
        for i in range(NCORES)
    ]
    res = run_bass_kernel_spmd(nc, in_maps, core_ids=list(range(NCORES)))
    return np.concatenate([r["out"] for r in res.results], axis=0)


if __name__ == "__main__":
    rng = np.random.default_rng(0)
    inputs = {
        "h": rng.standard_normal((B, L, IN), dtype=np.float32),
        "w": rng.standard_normal((IN, HID), dtype=np.float32),
        "u": rng.standard_normal((HID, SEQ), dtype=np.float32),
        "b": rng.standard_normal((1,), dtype=np.float32),
    }
    out = kernel(**inputs)
    print(out.shape, out.dtype)
